# revision 1
# baseline (speedup 1.0000x reference)
"""Causal self-attention (B=2, T=2048, C=1024, 16 heads x 64) on 8 TRN2 cores.

Sharding: tensor-parallel over heads (2 heads/core). Each core computes its
heads' QKV projection, causal attention, and a partial output projection
(contraction over its 128 attn columns); the host sums the 8 partials
(row-parallel all-reduce at gather time).

Per-core kernel layout (v2, PE-warmth/weight-reuse optimized):
  - x pre-transposed on host to xT [ci=128, co=8, B*T] (c = co*128+ci).
  - qT/kT/vT [f, t] computed with c-outer loops (stationary weight reused
    across 4 moving chunks); V additionally PE-transposed to t-major with
    an appended ones column.
  - Scores computed transposed, ST[k, q] = KT^T @ QT; the two heads' K=64
    matmuls are emitted alternately so they row-pack into the 128x128 PE
    array concurrently.
  - exp via one ACT pass per [128, 2, 512] group, PSUM->SBUF bf16.
  - No max-subtraction (scores ~N(0,1); exp safe in fp32).
  - PV accumulates [65, q]: V ones-column makes row 64 the softmax
    denominator l[q]. PV is ragged on the causal diagonal band.
  - Normalization (1/l) via reciprocal_approx_fast + K=2-style broadcast
    matmul (sel65) + one DVE multiply into attnT.
  - Output projection per q-chunk right after normalization (keeps PE
    busy through phase transitions); emits out[t, co] fp32 partials.
"""

import os

import numpy as np
import ml_dtypes

B = 2
T = 2048
C = 1024
N_HEADS = 16
D = 64
NCORES = 8
P = 128
BT = B * T
SCALE = D ** -0.5

_bf16 = ml_dtypes.bfloat16

_COMPILED = None
LAST_RESULTS = None  # stashed BassKernelResults for test harness introspection


def _build():
    import concourse.bass as bass
    import concourse.mybir as mybir
    import concourse.tile as tile
    from concourse import bacc

    f32 = mybir.dt.float32
    bf16 = mybir.dt.bfloat16

    nc = bacc.Bacc("TRN2", target_bir_lowering=False, debug=False,
                   num_devices=NCORES)

    xT_d = nc.dram_tensor("xT", [P, 8, BT], bf16, kind="ExternalInput")
    wqkvT_d = nc.dram_tensor("wqkvT", [P, 8, 384], bf16, kind="ExternalInput")
    woutT_d = nc.dram_tensor("woutT", [P, C], bf16, kind="ExternalInput")
    maskT_d = nc.dram_tensor("maskT", [P, P], bf16, kind="ExternalInput")
    sel2_d = nc.dram_tensor("sel2", [65, P], f32, kind="ExternalInput")
    ident_d = nc.dram_tensor("ident", [P, P], bf16, kind="ExternalInput")
    out_d = nc.dram_tensor("out", [BT, C], f32, kind="ExternalOutput")

    Exp = mybir.ActivationFunctionType.Exp

    with tile.TileContext(nc) as tc:
        with (
            tc.tile_pool(name="const", bufs=1) as const_pool,
            tc.tile_pool(name="xT", bufs=2) as xT_pool,
            tc.tile_pool(name="qkv", bufs=2) as qkv_pool,
            tc.tile_pool(name="pt", bufs=4) as pt_pool,
            tc.tile_pool(name="attnT", bufs=2) as attnT_pool,
            tc.tile_pool(name="rl", bufs=2) as rl_pool,
            tc.tile_pool(name="osb", bufs=3) as osb_pool,
            tc.tile_pool(name="st", bufs=2, space="PSUM") as st_pool,
            tc.tile_pool(name="ps4", bufs=4, space="PSUM") as ps4_pool,
        ):
            wqkvT = const_pool.tile([P, 8, 384], bf16, tag="wqkvT")
            woutT = const_pool.tile([P, C], bf16, tag="woutT")
            maskT = const_pool.tile([P, P], bf16, tag="maskT")
            sel2 = const_pool.tile([65, P], f32, tag="sel2")
            ident = const_pool.tile([P, P], bf16, tag="ident")
            nc.sync.dma_start(wqkvT, wqkvT_d[:])
            nc.sync.dma_start(woutT, woutT_d[:])
            nc.sync.dma_start(maskT, maskT_d[:])
            nc.sync.dma_start(sel2, sel2_d[:])
            nc.sync.dma_start(ident, ident_d[:])

            for b in range(B):
                xb = xT_pool.tile([P, 8, T], bf16, tag="xT")
                nc.sync.dma_start(xb, xT_d[:, :, b * T:(b + 1) * T])

                # ---- QKV projection: c-outer so the stationary weight is
                # reused across the 4 moving chunks of each projection.
                qT = qkv_pool.tile([P, T], bf16, tag="qT")
                kT = qkv_pool.tile([P, T], bf16, tag="kT")
                vT = qkv_pool.tile([P, T], bf16, tag="vT")
                for fi, dest in ((0, qT), (1, kT), (2, vT)):
                    pss = [ps4_pool.tile([P, 512], f32, tag="ps4",
                                         name=f"qkvps{n}")
                           for n in range(4)]
                    for c in range(8):
                        for n in range(4):
                            nc.tensor.matmul(
                                pss[n],
                                wqkvT[:, c, fi * 128:(fi + 1) * 128],
                                xb[:, c, n * 512:(n + 1) * 512],
                                start=(c == 0), stop=(c == 7),
                            )
                    for n in range(4):
                        nc.scalar.copy(dest[:, n * 512:(n + 1) * 512], pss[n])

                # V to t-major (PE transpose) with ones column appended.
                vh = [qkv_pool.tile([P, 16, 65], bf16, tag=f"v{h}",
                                    name=f"vh{h}")
                      for h in range(2)]
                for h in range(2):
                    nc.vector.memset(vh[h][:, :, 64], 1.0)
                for tch in range(16):
                    tp = ps4_pool.tile([P, P], bf16, tag="ps4", name="vtp")
                    nc.tensor.transpose(
                        tp, vT[:, tch * 128:(tch + 1) * 128], ident)
                    nc.scalar.copy(vh[0][:, tch, 0:64], tp[:, 0:64])
                    nc.scalar.copy(vh[1][:, tch, 0:64], tp[:, 64:128])

                # ---- attention (heads interleaved for PE row-packing) ----
                attnT = attnT_pool.tile([P, T], bf16, tag="attnT")
                rl2 = rl_pool.tile([65, T], f32, tag="rl2")
                l2 = rl_pool.tile([65, T], f32, tag="l2")
                # rows 1-63 feed zero sel2 rows; 1.0 keeps 1/x finite there
                nc.vector.memset(l2, 1.0)

                def norm_qc(qc):
                    # deps (recip) satisfied one qc ago -> no PE stall
                    qsl = slice(qc * 512, (qc + 1) * 512)
                    rb = ps4_pool.tile([P, 512], f32, tag="ps4", name="rb")
                    nc.tensor.matmul(rb, sel2[:, :], rl2[:, qsl],
                                     start=True, stop=True)
                    nc.vector.tensor_mul(attnT[:, qsl], attnT[:, qsl], rb)

                def oproj_tb(tb):
                    # PE filler: one token-block of the output projection
                    ps_a = ps4_pool.tile([P, 512], f32, tag="ps4",
                                         name="opa")
                    ps_b = ps4_pool.tile([P, 512], f32, tag="ps4",
                                         name="opb")
                    nc.tensor.matmul(
                        ps_a, attnT[:, tb * 128:(tb + 1) * 128],
                        woutT[:, 0:512], start=True, stop=True)
                    nc.tensor.matmul(
                        ps_b, attnT[:, tb * 128:(tb + 1) * 128],
                        woutT[:, 512:1024], start=True, stop=True)
                    osb = osb_pool.tile([P, C], f32, tag="osb")
                    nc.vector.tensor_copy(osb[:, 0:512], ps_a)
                    nc.vector.tensor_copy(osb[:, 512:1024], ps_b)
                    nc.sync.dma_start(
                        out_d[(b * T + tb * 128):(b * T + (tb + 1) * 128), :],
                        osb)

                for qc in range(4):
                    nk = 4 * qc + 4
                    qsl = slice(qc * 512, (qc + 1) * 512)
                    pv = [ps4_pool.tile([P, 512], f32, tag="ps4",
                                        name=f"pv{h}")
                          for h in range(2)]
                    if qc > 0:
                        norm_qc(qc - 1)
                    filler = list(range(4 * (qc - 1), 4 * qc)) if qc else []
                    for g0 in range(0, nk, 2):
                        kbs = list(range(g0, min(g0 + 2, nk)))
                        ng = len(kbs)
                        st = [st_pool.tile([P, 2, 512], f32, tag="st",
                                           name=f"st{h}")
                              for h in range(2)]
                        pt = [pt_pool.tile([P, 2, 512], bf16, tag="pt",
                                           name=f"pt{h}")
                              for h in range(2)]
                        # alternate heads so K=64 matmuls pack in the array
                        for j, kb in enumerate(kbs):
                            for h in range(2):
                                hs = h * 64
                                nc.tensor.matmul(
                                    st[h][:, j, :],
                                    kT[hs:hs + 64, kb * 128:(kb + 1) * 128],
                                    qT[hs:hs + 64, qsl],
                                    start=True, stop=True,
                                )
                        for h in range(2):
                            nc.scalar.activation(
                                pt[h][:, :ng, :], st[h][:, :ng, :], Exp,
                                scale=SCALE)
                        for j, kb in enumerate(kbs):
                            if kb >= 4 * qc:
                                off = (kb - 4 * qc) * 128
                                for h in range(2):
                                    nc.vector.tensor_mul(
                                        pt[h][:, j, off:off + 128],
                                        pt[h][:, j, off:off + 128],
                                        maskT,
                                    )
                        for j, kb in enumerate(kbs):
                            off = max(0, (kb - 4 * qc) * 128)
                            for h in range(2):
                                nc.tensor.matmul(
                                    pv[h][:65, off:512],
                                    vh[h][:, kb, :],
                                    pt[h][:, j, off:512],
                                    start=(kb == 0), stop=(kb == nk - 1),
                                    skip_group_check=True,
                                )
                        if filler:
                            oproj_tb(filler.pop(0))
                    # drain: denominators + unnormalized attnT.
                    # NOTE: custom-DVE ops (reciprocal_approx_*) mishandle
                    # non-zero partition bases on HW — move l to a base-0
                    # SBUF tile with regular copies first.
                    for h in range(2):
                        hs = h * 64
                        nc.vector.tensor_copy(
                            l2[hs:hs + 1, qsl], pv[h][64:65, :])
                        nc.vector.tensor_copy(
                            attnT[hs:hs + 64, qsl], pv[h][0:64, :])
                    nc.vector.reciprocal_approx_fast(
                        rl2[:, qsl], l2[:, qsl])
                    for tb in filler:
                        oproj_tb(tb)
                norm_qc(3)
                for tb in range(12, 16):
                    oproj_tb(tb)

    nc.compile()
    return nc


def _get_compiled():
    global _COMPILED
    if _COMPILED is None:
        _COMPILED = _build()
    return _COMPILED


def make_core_inputs(x, w_qkv, w_out):
    """Host-side shard prep: returns list of per-core input dicts."""
    xf = np.asarray(x, dtype=np.float32).reshape(BT, C)
    xT = np.ascontiguousarray(
        xf.T.reshape(8, P, BT).transpose(1, 0, 2)).astype(_bf16)

    maskT = np.zeros((P, P), dtype=_bf16)
    kk, qq = np.meshgrid(np.arange(P), np.arange(P), indexing="ij")
    maskT[kk <= qq] = 1.0

    sel2 = np.zeros((65, P), dtype=np.float32)
    sel2[0, 0:64] = 1.0
    sel2[64, 64:128] = 1.0

    ident = np.eye(P, dtype=_bf16)

    w_qkv = np.asarray(w_qkv, dtype=np.float32)
    w_out = np.asarray(w_out, dtype=np.float32)

    ins = []
    for core in range(NCORES):
        r0 = 2 * core * D
        wsel = np.concatenate([
            w_qkv[r0:r0 + 128],
            w_qkv[C + r0:C + r0 + 128],
            w_qkv[2 * C + r0:2 * C + r0 + 128],
        ], axis=0)  # [384, 1024]
        wqkvT = np.ascontiguousarray(
            wsel.T.reshape(8, P, 384).transpose(1, 0, 2)).astype(_bf16)
        woutT = np.ascontiguousarray(
            w_out[:, core * P:(core + 1) * P].T).astype(_bf16)
        ins.append({
            "xT": xT,
            "wqkvT": wqkvT,
            "woutT": woutT,
            "maskT": maskT,
            "sel2": sel2,
            "ident": ident,
        })
    return ins


def kernel(x, w_qkv, w_out):
    global LAST_RESULTS
    from concourse.bass_utils import run_bass_kernel_spmd

    nc = _get_compiled()
    ins = make_core_inputs(x, w_qkv, w_out)
    trace = bool(os.environ.get("KERNEL_TRACE"))
    res = run_bass_kernel_spmd(nc, ins, core_ids=list(range(NCORES)),
                               trace=trace)
    LAST_RESULTS = res
    out = np.zeros((BT, C), dtype=np.float32)
    for r in res.results:
        out += r["out"]
    return out.reshape(B, T, C)



# revision 9
# speedup vs baseline: 1.2381x; 1.2381x over previous
"""Causal self-attention (B=2, T=2048, C=1024, 16 heads x 64) on 8 TRN2 cores.

Sharding: tensor-parallel over heads (2 heads/core). Each core computes its
heads' QKV projection, causal attention, and a partial output projection
(contraction over its 128 attn columns); the host sums the 8 partials.

v3 design notes (HAM-warmth + engine-balance rewrite of v2):
  - PE warm-up: 6 dummy matmuls at t~0 so the HAM clock gate reaches 8/8
    (2.4 GHz) right as the first real matmuls issue.
  - x DMA'd in 4 token-chunks per batch; QKV runs token-chunk-pipelined
    (c-inner accumulation, one PSUM bank per chunk) so attention for batch 0
    starts ~6us in. Remaining chunks + all of batch 1's QKV/V-transposes are
    emitted as PE filler units INSIDE the attention loop (tensor-queue FIFO =
    schedule), keeping the PE dense so HAM never re-throttles.
  - Attention software pipeline: scores(g) -> exp(g) [scalar] -> mask(g)
    [gpsimd] -> PV(g), with PV emitted one group behind scores so the PE
    never head-blocks on the exp stream (the scalar engine's ~91us of exp is
    the attention-phase clock; it must never starve).
  - PV keeps the ones-column trick (ones column last -> PSUM row 64; DVE/ACT
    partition bases must be quadrant-aligned, and reciprocal_approx_* needs
    base 0, hence the l2 bounce).
  - Engine balance: exp on scalar; masks on gpsimd; PSUM evacuations split
    scalar/vector so neither exceeds the tensor engine's ~106us of matmul.
  - Drains of qc N are emitted after qc N+1's first exps so the scalar queue
    never head-blocks waiting for PV.
"""

import os
from collections import deque

import numpy as np
import ml_dtypes

B = 2
T = 2048
C = 1024
N_HEADS = 16
D = 64
NCORES = 8
P = 128
BT = B * T
SCALE = D ** -0.5
NCHUNK = 4          # token chunks per batch for QKV pipeline
CH = T // NCHUNK    # 512

_bf16 = ml_dtypes.bfloat16

_COMPILED = None
LAST_RESULTS = None  # stashed BassKernelResults for test harness introspection


def _build():
    import concourse.bass as bass
    import concourse.mybir as mybir
    import concourse.tile as tile
    from concourse import bacc

    f32 = mybir.dt.float32
    bf16 = mybir.dt.bfloat16

    nc = bacc.Bacc("TRN2", target_bir_lowering=False, debug=False,
                   num_devices=NCORES)

    xT_d = nc.dram_tensor("xT", [P, 8, BT], bf16, kind="ExternalInput")
    wqkvT_d = nc.dram_tensor("wqkvT", [P, 8, 384], bf16, kind="ExternalInput")
    woutT_d = nc.dram_tensor("woutT", [P, C], bf16, kind="ExternalInput")
    maskT_d = nc.dram_tensor("maskT", [P, P], bf16, kind="ExternalInput")
    sel_d = nc.dram_tensor("sel", [65, P], f32, kind="ExternalInput")
    ident_d = nc.dram_tensor("ident", [P, P], bf16, kind="ExternalInput")
    out_d = nc.dram_tensor("out", [BT, C], f32, kind="ExternalOutput")

    Exp = mybir.ActivationFunctionType.Exp

    with tile.TileContext(nc) as tc:
        with (
            tc.tile_pool(name="const", bufs=1) as const_pool,
            tc.tile_pool(name="xb", bufs=2) as xb_pool,
            tc.tile_pool(name="qkv", bufs=2) as qkv_pool,
            tc.tile_pool(name="pt", bufs=4) as pt_pool,
            tc.tile_pool(name="attnT", bufs=2) as attnT_pool,
            tc.tile_pool(name="rl", bufs=2) as rl_pool,
            tc.tile_pool(name="osb", bufs=3) as osb_pool,
            tc.tile_pool(name="st", bufs=2, space="PSUM") as st_pool,
            tc.tile_pool(name="pv", bufs=1, space="PSUM") as pv_pool,
            tc.tile_pool(name="ps", bufs=2, space="PSUM") as ps_pool,
        ):
            # ---- constants ----
            wqkvT = const_pool.tile([P, 8, 384], bf16, tag="wqkvT")
            woutT = const_pool.tile([P, C], bf16, tag="woutT")
            maskT = const_pool.tile([P, P], bf16, tag="maskT")
            sel = const_pool.tile([65, P], f32, tag="sel")
            ident = const_pool.tile([P, P], bf16, tag="ident")
            dummy = const_pool.tile([P, 512], bf16, tag="dummy")
            nc.sync.dma_start(wqkvT, wqkvT_d[:])
            nc.sync.dma_start(ident, ident_d[:])
            nc.sync.dma_start(maskT, maskT_d[:])
            nc.sync.dma_start(sel, sel_d[:])
            nc.sync.dma_start(woutT, woutT_d[:])

            # ---- x, token-chunked ----
            xb = []
            for b in range(B):
                xt = xb_pool.tile([P, 8, T], bf16, tag="xb", name=f"xb{b}")
                xb.append(xt)
                for n in range(NCHUNK):
                    nc.sync.dma_start(
                        xt[:, :, n * CH:(n + 1) * CH],
                        xT_d[:, :, b * T + n * CH:b * T + (n + 1) * CH])

            # ---- PE warm-up (HAM: reach K=8/8 before real matmuls) ----
            nc.vector.memset(dummy, 0.0)
            wm = ps_pool.tile([P, 512], f32, tag="ps", name="wm")
            for _ in range(6):
                nc.tensor.matmul(wm, dummy[:, 0:128], dummy[:, 0:512],
                                 start=True, stop=True)

            # ---- per-batch working tiles ----
            qT, kT, vT, vh, attnT, rl2, l2 = [], [], [], [], [], [], []
            for b in range(B):
                qT.append(qkv_pool.tile([P, T], bf16, tag="qT", name=f"qT{b}"))
                kT.append(qkv_pool.tile([P, T], bf16, tag="kT", name=f"kT{b}"))
                vT.append(qkv_pool.tile([P, T], bf16, tag="vT", name=f"vT{b}"))
                vh.append(qkv_pool.tile([P, 16, 2, 65], bf16, tag="vh",
                                        name=f"vh{b}"))
                attnT.append(attnT_pool.tile([P, T], bf16, tag="attnT",
                                             name=f"attnT{b}"))
                rl2.append(rl_pool.tile([65, T], f32, tag="rl2",
                                        name=f"rl2{b}"))
                l2.append(rl_pool.tile([65, T], f32, tag="l2",
                                       name=f"l2{b}"))
                # rows 1-63 are never written; keep them finite for recip
                nc.vector.memset(l2[b], 1.0)
                # ones column (col 64) -> PV row 64 = softmax denominator
                nc.vector.memset(vh[b][:, :, :, 64], 1.0)

            # ---------- emission units ----------
            def emit_qkv(b, n, fi, copy_fn):
                """One projection (fi: 0=q,1=k,2=v) for token chunk n."""
                dest = (qT, kT, vT)[fi][b]
                nsl = slice(n * CH, (n + 1) * CH)
                pss = ps_pool.tile([P, 512], f32, tag="ps", name="pss")
                for c in range(8):
                    nc.tensor.matmul(
                        pss, wqkvT[:, c, fi * 128:(fi + 1) * 128],
                        xb[b][:, c, nsl], start=(c == 0), stop=(c == 7))
                copy_fn(dest[:, nsl], pss)

            def emit_transposes(b, n):
                """PE-transpose V token chunk n (4 t-blocks) into vh."""
                tp = ps_pool.tile([P, 4, P], bf16, tag="ps", name="tp")
                for j in range(4):
                    tb = 4 * n + j
                    nc.tensor.transpose(
                        tp[:, j, :], vT[b][:, tb * 128:(tb + 1) * 128], ident)
                for h in range(2):
                    nc.vector.tensor_copy(
                        vh[b][:, 4 * n:4 * n + 4, h, 0:64],
                        tp[:, :, h * 64:(h + 1) * 64])

            def emit_oproj(b, tb):
                ps_a = ps_pool.tile([P, 512], f32, tag="ps", name="opa")
                ps_b = ps_pool.tile([P, 512], f32, tag="ps", name="opb")
                tsl = slice(tb * 128, (tb + 1) * 128)
                nc.tensor.matmul(ps_a, attnT[b][:, tsl], woutT[:, 0:512],
                                 start=True, stop=True)
                nc.tensor.matmul(ps_b, attnT[b][:, tsl], woutT[:, 512:1024],
                                 start=True, stop=True)
                osb = osb_pool.tile([P, C], f32, tag="osb")
                nc.vector.tensor_copy(osb[:, 0:512], ps_a)
                nc.vector.tensor_copy(osb[:, 512:1024], ps_b)
                nc.sync.dma_start(
                    out_d[(b * T + tb * 128):(b * T + (tb + 1) * 128), :], osb)

            def emit_rbnorm(b, qc):
                qsl = slice(qc * 512, (qc + 1) * 512)
                rb = ps_pool.tile([P, 512], f32, tag="ps", name="rb")
                nc.tensor.matmul(rb, sel[:, :], rl2[b][:, qsl],
                                 start=True, stop=True)
                nc.vector.tensor_mul(attnT[b][:, qsl], attnT[b][:, qsl], rb)

            # filler queue: popped one unit per site inside the attention loop
            F = deque()

            def pop_F():
                if F:
                    F.popleft()()

            # filler order: V-transposes and remaining QKV chunks first (they
            # have deadlines inside batch-0 attention), then batch 1's QKV,
            # then (appended later, at qc ends) rbnorm+oproj units.
            F.append(lambda: emit_transposes(0, 0))
            for n in range(1, NCHUNK):
                for fi in (1, 0, 2):
                    F.append(lambda b=0, n=n, fi=fi:
                             emit_qkv(b, n, fi, nc.vector.tensor_copy))
                F.append(lambda n=n: emit_transposes(0, n))
            for n in range(NCHUNK):
                for fi in (1, 0, 2):
                    F.append(lambda b=1, n=n, fi=fi:
                             emit_qkv(b, n, fi, nc.vector.tensor_copy))
                F.append(lambda n=n: emit_transposes(1, n))

            # ---------- lead-in: batch 0 token chunk 0 ----------
            for fi in (1, 0, 2):
                emit_qkv(0, 0, fi, nc.scalar.copy)

            # ---------- attention ----------
            pending_drain = [None]

            def make_drain(b, qc):
                qsl = slice(qc * 512, (qc + 1) * 512)

                def drain(pv):
                    # reciprocal_approx_* requires partition base 0: bounce
                    # the two denominator rows into l2 first.
                    for h in range(2):
                        nc.vector.tensor_copy(l2[b][64 * h:64 * h + 1, qsl],
                                              pv[h][64:65, :])
                    nc.vector.reciprocal_approx_fast(rl2[b][:, qsl],
                                                     l2[b][:, qsl])
                    nc.scalar.copy(attnT[b][0:64, qsl], pv[0][0:64, :])
                    nc.vector.tensor_copy(attnT[b][64:128, qsl],
                                          pv[1][0:64, :])
                    F.appendleft(lambda: emit_rbnorm(b, qc))
                    for tb in range(4 * qc, 4 * qc + 4):
                        F.append(lambda b=b, tb=tb: emit_oproj(b, tb))
                return drain

            def attention(b):
                for qc in range(4):
                    nk = 4 * qc + 4
                    qsl = slice(qc * 512, (qc + 1) * 512)
                    pv = [pv_pool.tile([P, 512], f32, tag=f"pv{h}",
                                       name=f"pv{h}")
                          for h in range(2)]
                    prev = None
                    for g0 in range(0, nk, 2):
                        kbs = [g0, g0 + 1]
                        st = [st_pool.tile([P, 2, 512], f32, tag="st",
                                           name=f"st{h}")
                              for h in range(2)]
                        pt = [pt_pool.tile([P, 2, 512], bf16, tag="pt",
                                           name=f"pt{h}")
                              for h in range(2)]
                        # scores, heads interleaved for PE row-packing
                        for j, kb in enumerate(kbs):
                            for h in range(2):
                                hs = h * 64
                                nc.tensor.matmul(
                                    st[h][:, j, :],
                                    kT[b][hs:hs + 64,
                                          kb * 128:(kb + 1) * 128],
                                    qT[b][hs:hs + 64, qsl],
                                    start=True, stop=True)
                        for h in range(2):
                            nc.scalar.activation(pt[h], st[h], Exp,
                                                 scale=SCALE)
                        if g0 == 0 and pending_drain[0] is not None:
                            pending_drain[0]()
                            pending_drain[0] = None
                        # causal masks on the diagonal blocks (gpsimd)
                        for j, kb in enumerate(kbs):
                            if kb >= 4 * qc:
                                off = (kb - 4 * qc) * 128
                                for h in range(2):
                                    nc.gpsimd.tensor_mul(
                                        pt[h][:, j, off:off + 128],
                                        pt[h][:, j, off:off + 128],
                                        maskT)
                        pop_F()
                        if prev is not None:
                            emit_pv(b, qc, nk, pv, *prev)
                        pop_F()
                        prev = (pt, kbs)
                    emit_pv(b, qc, nk, pv, *prev)
                    dr = make_drain(b, qc)
                    pending_drain[0] = lambda dr=dr, pv=pv: dr(pv)
                    pop_F()

            def emit_pv(b, qc, nk, pv, pt, kbs):
                for h in range(2):
                    for j, kb in enumerate(kbs):
                        off = max(0, (kb - 4 * qc) * 128)
                        nc.tensor.matmul(
                            pv[h][:65, off:512],
                            vh[b][:, kb, h, :],
                            pt[h][:, j, off:512],
                            start=(kb == 0), stop=(kb == nk - 1),
                            skip_group_check=True)

            attention(0)
            attention(1)
            if pending_drain[0] is not None:
                pending_drain[0]()
                pending_drain[0] = None
            while F:
                pop_F()

    nc.compile()
    return nc


def _get_compiled():
    global _COMPILED
    if _COMPILED is None:
        _COMPILED = _build()
    return _COMPILED


def make_core_inputs(x, w_qkv, w_out):
    """Host-side shard prep: returns list of per-core input dicts."""
    xf = np.asarray(x, dtype=np.float32).reshape(BT, C)
    xT = np.ascontiguousarray(
        xf.T.reshape(8, P, BT).transpose(1, 0, 2)).astype(_bf16)

    maskT = np.zeros((P, P), dtype=_bf16)
    kk, qq = np.meshgrid(np.arange(P), np.arange(P), indexing="ij")
    maskT[kk <= qq] = 1.0

    sel = np.zeros((65, P), dtype=np.float32)
    sel[0, 0:64] = 1.0
    sel[64, 64:128] = 1.0

    ident = np.eye(P, dtype=_bf16)

    w_qkv = np.asarray(w_qkv, dtype=np.float32)
    w_out = np.asarray(w_out, dtype=np.float32)

    ins = []
    for core in range(NCORES):
        r0 = 2 * core * D
        wsel = np.concatenate([
            w_qkv[r0:r0 + 128],
            w_qkv[C + r0:C + r0 + 128],
            w_qkv[2 * C + r0:2 * C + r0 + 128],
        ], axis=0)  # [384, 1024]
        wqkvT = np.ascontiguousarray(
            wsel.T.reshape(8, P, 384).transpose(1, 0, 2)).astype(_bf16)
        woutT = np.ascontiguousarray(
            w_out[:, core * P:(core + 1) * P].T).astype(_bf16)
        ins.append({
            "xT": xT,
            "wqkvT": wqkvT,
            "woutT": woutT,
            "maskT": maskT,
            "sel": sel,
            "ident": ident,
        })
    return ins


def kernel(x, w_qkv, w_out):
    global LAST_RESULTS
    from concourse.bass_utils import run_bass_kernel_spmd

    nc = _get_compiled()
    ins = make_core_inputs(x, w_qkv, w_out)
    trace = bool(os.environ.get("KERNEL_TRACE"))
    res = run_bass_kernel_spmd(nc, ins, core_ids=list(range(NCORES)),
                               trace=trace)
    LAST_RESULTS = res
    out = np.zeros((BT, C), dtype=np.float32)
    for r in res.results:
        out += r["out"]
    return out.reshape(B, T, C)


# revision 10
# speedup vs baseline: 1.2414x; 1.0026x over previous
"""Causal self-attention (B=2, T=2048, C=1024, 16 heads x 64) on 8 TRN2 cores.

Sharding: tensor-parallel over heads (2 heads/core). Each core computes its
heads' QKV projection, causal attention, and a partial output projection
(contraction over its 128 attn columns); the host sums the 8 partials.

v3 design notes (HAM-warmth + engine-balance rewrite of v2):
  - PE warm-up: 6 dummy matmuls at t~0 so the HAM clock gate reaches 8/8
    (2.4 GHz) right as the first real matmuls issue.
  - x DMA'd in 4 token-chunks per batch; QKV runs token-chunk-pipelined
    (c-inner accumulation, one PSUM bank per chunk) so attention for batch 0
    starts ~6us in. Remaining chunks + all of batch 1's QKV/V-transposes are
    emitted as PE filler units INSIDE the attention loop (tensor-queue FIFO =
    schedule), keeping the PE dense so HAM never re-throttles.
  - Attention software pipeline: scores(g) -> exp(g) [scalar] -> mask(g)
    [gpsimd] -> PV(g), with PV emitted one group behind scores so the PE
    never head-blocks on the exp stream (the scalar engine's ~91us of exp is
    the attention-phase clock; it must never starve).
  - PV keeps the ones-column trick (ones column last -> PSUM row 64; DVE/ACT
    partition bases must be quadrant-aligned, and reciprocal_approx_* needs
    base 0, hence the l2 bounce).
  - Engine balance: exp on scalar; masks on gpsimd; PSUM evacuations split
    scalar/vector so neither exceeds the tensor engine's ~106us of matmul.
  - Drains of qc N are emitted after qc N+1's first exps so the scalar queue
    never head-blocks waiting for PV.
"""

import os
from collections import deque

import numpy as np
import ml_dtypes

B = 2
T = 2048
C = 1024
N_HEADS = 16
D = 64
NCORES = 8
P = 128
BT = B * T
SCALE = D ** -0.5
NCHUNK = 4          # token chunks per batch for QKV pipeline
CH = T // NCHUNK    # 512

_bf16 = ml_dtypes.bfloat16

_COMPILED = None
LAST_RESULTS = None  # stashed BassKernelResults for test harness introspection


def _build():
    import concourse.bass as bass
    import concourse.mybir as mybir
    import concourse.tile as tile
    from concourse import bacc

    f32 = mybir.dt.float32
    bf16 = mybir.dt.bfloat16

    nc = bacc.Bacc("TRN2", target_bir_lowering=False, debug=False,
                   num_devices=NCORES)

    xT_d = nc.dram_tensor("xT", [P, B, NCHUNK, 8, CH], bf16,
                          kind="ExternalInput")
    wqkvT_d = nc.dram_tensor("wqkvT", [P, 8, 384], bf16, kind="ExternalInput")
    woutT_d = nc.dram_tensor("woutT", [P, C], bf16, kind="ExternalInput")
    btri_d = nc.dram_tensor("btri", [P, P], bf16, kind="ExternalInput")
    sel_d = nc.dram_tensor("sel", [65, P], f32, kind="ExternalInput")
    ident_d = nc.dram_tensor("ident", [P, P], bf16, kind="ExternalInput")
    out_d = nc.dram_tensor("out", [BT, C], f32, kind="ExternalOutput")

    Exp = mybir.ActivationFunctionType.Exp

    with tile.TileContext(nc) as tc:
        with (
            tc.tile_pool(name="const", bufs=1) as const_pool,
            tc.tile_pool(name="xb", bufs=2) as xb_pool,
            tc.tile_pool(name="qkv", bufs=2) as qkv_pool,
            tc.tile_pool(name="pt", bufs=4) as pt_pool,
            tc.tile_pool(name="attnT", bufs=2) as attnT_pool,
            tc.tile_pool(name="rl", bufs=2) as rl_pool,
            tc.tile_pool(name="osb", bufs=3) as osb_pool,
            tc.tile_pool(name="st", bufs=2, space="PSUM") as st_pool,
            tc.tile_pool(name="pv", bufs=1, space="PSUM") as pv_pool,
            tc.tile_pool(name="ps", bufs=2, space="PSUM") as ps_pool,
        ):
            # ---- constants ----
            wqkvT = const_pool.tile([P, 8, 384], bf16, tag="wqkvT")
            woutT = const_pool.tile([P, C], bf16, tag="woutT")
            btri = const_pool.tile([P, P], bf16, tag="btri")
            sel = const_pool.tile([65, P], f32, tag="sel")
            ident = const_pool.tile([P, P], bf16, tag="ident")
            dummy = const_pool.tile([P, 512], bf16, tag="dummy")
            # K weights + first x chunk first: they gate the first real matmul
            nc.sync.dma_start(wqkvT[:, :, 128:256], wqkvT_d[:, :, 128:256])

            # ---- x, token-chunked (batch 1 chunks deferred into the
            # attention filler stream to keep early HBM read BW for batch 0)
            xb = []
            for b in range(B):
                xt = xb_pool.tile([P, NCHUNK, 8, CH], bf16, tag="xb",
                                  name=f"xb{b}")
                xb.append(xt)
            nc.sync.dma_start(xb[0][:, 0], xT_d[:, 0, 0])
            nc.sync.dma_start(wqkvT[:, :, 0:128], wqkvT_d[:, :, 0:128])
            nc.sync.dma_start(wqkvT[:, :, 256:384], wqkvT_d[:, :, 256:384])
            for n in range(1, NCHUNK):
                nc.sync.dma_start(xb[0][:, n], xT_d[:, 0, n])
            nc.sync.dma_start(ident, ident_d[:])
            nc.sync.dma_start(btri, btri_d[:])
            nc.sync.dma_start(sel, sel_d[:])
            nc.sync.dma_start(woutT, woutT_d[:])

            # ---- PE warm-up (HAM: reach K=8/8 before real matmuls) ----
            nc.vector.memset(dummy, 0.0)
            wm = ps_pool.tile([P, 512], f32, tag="ps", name="wm")
            for _ in range(10):
                nc.tensor.matmul(wm, dummy[:, 0:128], dummy[:, 0:512],
                                 start=True, stop=True)

            # ---- per-batch working tiles ----
            qT, kT, vT, vh, attnT, rl2, l2 = [], [], [], [], [], [], []
            for b in range(B):
                qT.append(qkv_pool.tile([P, T], bf16, tag="qT", name=f"qT{b}"))
                kT.append(qkv_pool.tile([P, T], bf16, tag="kT", name=f"kT{b}"))
                vT.append(qkv_pool.tile([P, T], bf16, tag="vT", name=f"vT{b}"))
                vh.append(qkv_pool.tile([P, 16, 2, 65], bf16, tag="vh",
                                        name=f"vh{b}"))
                attnT.append(attnT_pool.tile([P, T], bf16, tag="attnT",
                                             name=f"attnT{b}"))
                rl2.append(rl_pool.tile([65, T], f32, tag="rl2",
                                        name=f"rl2{b}"))
                l2.append(rl_pool.tile([65, T], f32, tag="l2",
                                       name=f"l2{b}"))
                # rows 1-63 are never written; keep them finite for recip
                nc.vector.memset(l2[b], 1.0)
                # ones column (col 64) -> PV row 64 = softmax denominator
                nc.vector.memset(vh[b][:, :, :, 64], 1.0)

            # ---------- emission units ----------
            def emit_qkv(b, n, fi, copy_fn):
                """One projection (fi: 0=q,1=k,2=v) for token chunk n."""
                dest = (qT, kT, vT)[fi][b]
                nsl = slice(n * CH, (n + 1) * CH)
                pss = ps_pool.tile([P, 512], f32, tag="ps", name="pss")
                for c in range(8):
                    nc.tensor.matmul(
                        pss, wqkvT[:, c, fi * 128:(fi + 1) * 128],
                        xb[b][:, n, c, :], start=(c == 0), stop=(c == 7))
                copy_fn(dest[:, nsl], pss)

            def emit_transposes(b, n):
                """PE-transpose V token chunk n (4 t-blocks) into vh."""
                tp = ps_pool.tile([P, 4, P], bf16, tag="ps", name="tp")
                for j in range(4):
                    tb = 4 * n + j
                    nc.tensor.transpose(
                        tp[:, j, :], vT[b][:, tb * 128:(tb + 1) * 128], ident)
                for h in range(2):
                    nc.vector.tensor_copy(
                        vh[b][:, 4 * n:4 * n + 4, h, 0:64],
                        tp[:, :, h * 64:(h + 1) * 64])

            def emit_oproj(b, tb):
                ps_a = ps_pool.tile([P, 512], f32, tag="ps", name="opa")
                ps_b = ps_pool.tile([P, 512], f32, tag="ps", name="opb")
                tsl = slice(tb * 128, (tb + 1) * 128)
                nc.tensor.matmul(ps_a, attnT[b][:, tsl], woutT[:, 0:512],
                                 start=True, stop=True)
                nc.tensor.matmul(ps_b, attnT[b][:, tsl], woutT[:, 512:1024],
                                 start=True, stop=True)
                osb = osb_pool.tile([P, C], f32, tag="osb")
                nc.scalar.copy(osb[:, 0:512], ps_a)
                nc.vector.tensor_copy(osb[:, 512:1024], ps_b)
                nc.gpsimd.dma_start(
                    out_d[(b * T + tb * 128):(b * T + (tb + 1) * 128), :], osb)

            def emit_rbnorm(b, qc):
                qsl = slice(qc * 512, (qc + 1) * 512)
                rb = ps_pool.tile([P, 512], f32, tag="ps", name="rb")
                nc.tensor.matmul(rb, sel[:, :], rl2[b][:, qsl],
                                 start=True, stop=True)
                nc.vector.tensor_mul(attnT[b][:, qsl], attnT[b][:, qsl], rb)

            # filler queue: popped one unit per site inside the attention loop
            F = deque()

            def pop_F():
                if F:
                    F.popleft()()

            # filler order: V-transposes and remaining QKV chunks first (they
            # have deadlines inside batch-0 attention), then batch 1's QKV,
            # then (appended later, at qc ends) rbnorm+oproj units.
            F.append(lambda: emit_transposes(0, 0))
            for n in range(1, NCHUNK):
                F.append(lambda n=n: nc.sync.dma_start(xb[1][:, n - 1],
                                                       xT_d[:, 1, n - 1]))
                for fi in (1, 0, 2):
                    F.append(lambda b=0, n=n, fi=fi:
                             emit_qkv(b, n, fi, nc.vector.tensor_copy))
                F.append(lambda n=n: emit_transposes(0, n))
            F.append(lambda: nc.sync.dma_start(xb[1][:, NCHUNK - 1],
                                               xT_d[:, 1, NCHUNK - 1]))
            for n in range(NCHUNK):
                for fi in (1, 0, 2):
                    F.append(lambda b=1, n=n, fi=fi:
                             emit_qkv(b, n, fi, nc.vector.tensor_copy))
                F.append(lambda n=n: emit_transposes(1, n))

            # ---------- lead-in: batch 0 token chunk 0 ----------
            for fi in (1, 0, 2):
                emit_qkv(0, 0, fi, nc.scalar.copy)

            # ---------- attention ----------
            pending_drain = [None]

            def make_drain(b, qc):
                qsl = slice(qc * 512, (qc + 1) * 512)

                def drain(pv):
                    # reciprocal_approx_* requires partition base 0: bounce
                    # the two denominator rows into l2 first.
                    for h in range(2):
                        nc.vector.tensor_copy(l2[b][64 * h:64 * h + 1, qsl],
                                              pv[h][64:65, :])
                    nc.vector.reciprocal_approx_fast(rl2[b][:, qsl],
                                                     l2[b][:, qsl])
                    nc.scalar.copy(attnT[b][0:64, qsl], pv[0][0:64, :])
                    nc.vector.tensor_copy(attnT[b][64:128, qsl],
                                          pv[1][0:64, :])
                    F.appendleft(lambda: emit_rbnorm(b, qc))
                    for tb in range(4 * qc, 4 * qc + 4):
                        F.append(lambda b=b, tb=tb: emit_oproj(b, tb))
                return drain

            def attention(b):
                for qc in range(4):
                    nk = 4 * qc + 4
                    qsl = slice(qc * 512, (qc + 1) * 512)
                    pv = [pv_pool.tile([P, 512], f32, tag=f"pv{h}",
                                       name=f"pv{h}")
                          for h in range(2)]
                    prev = None
                    for g0 in range(0, nk, 2):
                        kbs = [g0, g0 + 1]
                        st = [st_pool.tile([P, 2, 512], f32, tag="st",
                                           name=f"st{h}")
                              for h in range(2)]
                        pt = [pt_pool.tile([P, 2, 512], bf16, tag="pt",
                                           name=f"pt{h}")
                              for h in range(2)]
                        # scores, heads interleaved for PE row-packing;
                        # diagonal blocks get the causal mask accumulated in
                        # as a -1e4 upper-triangle bias matmul (exp -> 0)
                        for j, kb in enumerate(kbs):
                            diag = kb >= 4 * qc
                            off = (kb - 4 * qc) * 128
                            for h in range(2):
                                hs = h * 64
                                nc.tensor.matmul(
                                    st[h][:, j, :],
                                    kT[b][hs:hs + 64,
                                          kb * 128:(kb + 1) * 128],
                                    qT[b][hs:hs + 64, qsl],
                                    start=True, stop=not diag)
                            if diag:
                                for h in range(2):
                                    nc.tensor.matmul(
                                        st[h][:, j, off:off + 128],
                                        ident, btri,
                                        start=False, stop=True,
                                        skip_group_check=True)
                        for h in range(2):
                            nc.scalar.activation(pt[h], st[h], Exp,
                                                 scale=SCALE)
                        if g0 == 0 and pending_drain[0] is not None:
                            pending_drain[0]()
                            pending_drain[0] = None
                        pop_F()
                        if prev is not None:
                            emit_pv(b, qc, nk, pv, *prev)
                        pop_F()
                        prev = (pt, kbs)
                    emit_pv(b, qc, nk, pv, *prev)
                    dr = make_drain(b, qc)
                    pending_drain[0] = lambda dr=dr, pv=pv: dr(pv)
                    pop_F()

            def emit_pv(b, qc, nk, pv, pt, kbs):
                for h in range(2):
                    for j, kb in enumerate(kbs):
                        off = max(0, (kb - 4 * qc) * 128)
                        nc.tensor.matmul(
                            pv[h][:65, off:512],
                            vh[b][:, kb, h, :],
                            pt[h][:, j, off:512],
                            start=(kb == 0), stop=(kb == nk - 1),
                            skip_group_check=True)

            attention(0)
            attention(1)
            if pending_drain[0] is not None:
                pending_drain[0]()
                pending_drain[0] = None
            while F:
                pop_F()

    nc.compile()
    return nc


def _get_compiled():
    global _COMPILED
    if _COMPILED is None:
        _COMPILED = _build()
    return _COMPILED


def make_core_inputs(x, w_qkv, w_out):
    """Host-side shard prep: returns list of per-core input dicts."""
    xf = np.asarray(x, dtype=np.float32).reshape(BT, C)
    # xT[ci, b, n, co, t] = x[b, n*CH + t, co*128 + ci]
    xT = np.ascontiguousarray(
        xf.T.reshape(8, P, B, NCHUNK, CH).transpose(1, 2, 3, 0, 4)
    ).astype(_bf16)

    btri = np.zeros((P, P), dtype=np.float32)
    kk, qq = np.meshgrid(np.arange(P), np.arange(P), indexing="ij")
    btri[kk > qq] = -1e4
    btri = btri.astype(_bf16)

    sel = np.zeros((65, P), dtype=np.float32)
    sel[0, 0:64] = 1.0
    sel[64, 64:128] = 1.0

    ident = np.eye(P, dtype=_bf16)

    w_qkv = np.asarray(w_qkv, dtype=np.float32)
    w_out = np.asarray(w_out, dtype=np.float32)

    ins = []
    for core in range(NCORES):
        r0 = 2 * core * D
        wsel = np.concatenate([
            w_qkv[r0:r0 + 128],
            w_qkv[C + r0:C + r0 + 128],
            w_qkv[2 * C + r0:2 * C + r0 + 128],
        ], axis=0)  # [384, 1024]
        wqkvT = np.ascontiguousarray(
            wsel.T.reshape(8, P, 384).transpose(1, 0, 2)).astype(_bf16)
        woutT = np.ascontiguousarray(
            w_out[:, core * P:(core + 1) * P].T).astype(_bf16)
        ins.append({
            "xT": xT,
            "wqkvT": wqkvT,
            "woutT": woutT,
            "btri": btri,
            "sel": sel,
            "ident": ident,
        })
    return ins


def kernel(x, w_qkv, w_out):
    global LAST_RESULTS
    from concourse.bass_utils import run_bass_kernel_spmd

    nc = _get_compiled()
    ins = make_core_inputs(x, w_qkv, w_out)
    trace = bool(os.environ.get("KERNEL_TRACE"))
    res = run_bass_kernel_spmd(nc, ins, core_ids=list(range(NCORES)),
                               trace=trace)
    LAST_RESULTS = res
    out = np.zeros((BT, C), dtype=np.float32)
    for r in res.results:
        out += r["out"]
    return out.reshape(B, T, C)


# revision 11
# speedup vs baseline: 1.2570x; 1.0126x over previous
"""Causal self-attention (B=2, T=2048, C=1024, 16 heads x 64) on 8 TRN2 cores.

Sharding: tensor-parallel over heads (2 heads/core). Each core computes its
heads' QKV projection, causal attention, and a partial output projection
(contraction over its 128 attn columns); the host sums the 8 partials.

v3 design notes (HAM-warmth + engine-balance rewrite of v2):
  - PE warm-up: 6 dummy matmuls at t~0 so the HAM clock gate reaches 8/8
    (2.4 GHz) right as the first real matmuls issue.
  - x DMA'd in 4 token-chunks per batch; QKV runs token-chunk-pipelined
    (c-inner accumulation, one PSUM bank per chunk) so attention for batch 0
    starts ~6us in. Remaining chunks + all of batch 1's QKV/V-transposes are
    emitted as PE filler units INSIDE the attention loop (tensor-queue FIFO =
    schedule), keeping the PE dense so HAM never re-throttles.
  - Attention software pipeline: scores(g) -> exp(g) [scalar] -> mask(g)
    [gpsimd] -> PV(g), with PV emitted one group behind scores so the PE
    never head-blocks on the exp stream (the scalar engine's ~91us of exp is
    the attention-phase clock; it must never starve).
  - PV keeps the ones-column trick (ones column last -> PSUM row 64; DVE/ACT
    partition bases must be quadrant-aligned, and reciprocal_approx_* needs
    base 0, hence the l2 bounce).
  - Engine balance: exp on scalar; masks on gpsimd; PSUM evacuations split
    scalar/vector so neither exceeds the tensor engine's ~106us of matmul.
  - Drains of qc N are emitted after qc N+1's first exps so the scalar queue
    never head-blocks waiting for PV.
"""

import os
from collections import deque

import numpy as np
import ml_dtypes

B = 2
T = 2048
C = 1024
N_HEADS = 16
D = 64
NCORES = 8
P = 128
BT = B * T
SCALE = D ** -0.5
NCHUNK = 4          # token chunks per batch for QKV pipeline
CH = T // NCHUNK    # 512

_bf16 = ml_dtypes.bfloat16

_COMPILED = None
LAST_RESULTS = None  # stashed BassKernelResults for test harness introspection


def _build():
    import concourse.bass as bass
    import concourse.mybir as mybir
    import concourse.tile as tile
    from concourse import bacc

    f32 = mybir.dt.float32
    bf16 = mybir.dt.bfloat16

    nc = bacc.Bacc("TRN2", target_bir_lowering=False, debug=False,
                   num_devices=NCORES)

    xT_d = nc.dram_tensor("xT", [P, B, NCHUNK, 8, CH], bf16,
                          kind="ExternalInput")
    wqkvT_d = nc.dram_tensor("wqkvT", [P, 8, 384], bf16, kind="ExternalInput")
    woutT_d = nc.dram_tensor("woutT", [P, C], bf16, kind="ExternalInput")
    btri_d = nc.dram_tensor("btri", [P, P], bf16, kind="ExternalInput")
    sel_d = nc.dram_tensor("sel", [65, P], f32, kind="ExternalInput")
    ident_d = nc.dram_tensor("ident", [P, P], bf16, kind="ExternalInput")
    out_d = nc.dram_tensor("out", [BT, C], f32, kind="ExternalOutput")

    Exp = mybir.ActivationFunctionType.Exp

    with tile.TileContext(nc) as tc:
        with (
            tc.tile_pool(name="const", bufs=1) as const_pool,
            tc.tile_pool(name="xb", bufs=2) as xb_pool,
            tc.tile_pool(name="qkv", bufs=2) as qkv_pool,
            tc.tile_pool(name="pt", bufs=4) as pt_pool,
            tc.tile_pool(name="attnT", bufs=2) as attnT_pool,
            tc.tile_pool(name="rl", bufs=2) as rl_pool,
            tc.tile_pool(name="osb", bufs=3) as osb_pool,
            tc.tile_pool(name="st", bufs=2, space="PSUM") as st_pool,
            tc.tile_pool(name="pv", bufs=1, space="PSUM") as pv_pool,
            tc.tile_pool(name="ps", bufs=2, space="PSUM") as ps_pool,
        ):
            # ---- constants ----
            wqkvT = const_pool.tile([P, 8, 384], bf16, tag="wqkvT")
            woutT = const_pool.tile([P, C], bf16, tag="woutT")
            btri = const_pool.tile([P, P], bf16, tag="btri")
            sel = const_pool.tile([65, P], f32, tag="sel")
            ident = const_pool.tile([P, P], bf16, tag="ident")
            dummy = const_pool.tile([P, 512], bf16, tag="dummy")
            # K weights + first x chunk first: they gate the first real matmul
            nc.sync.dma_start(wqkvT[:, :, 128:256], wqkvT_d[:, :, 128:256])

            # ---- x, token-chunked (batch 1 chunks deferred into the
            # attention filler stream to keep early HBM read BW for batch 0)
            xb = []
            for b in range(B):
                xt = xb_pool.tile([P, NCHUNK, 8, CH], bf16, tag="xb",
                                  name=f"xb{b}")
                xb.append(xt)
            nc.sync.dma_start(xb[0][:, 0], xT_d[:, 0, 0])
            nc.sync.dma_start(wqkvT[:, :, 0:128], wqkvT_d[:, :, 0:128])
            nc.sync.dma_start(wqkvT[:, :, 256:384], wqkvT_d[:, :, 256:384])
            for n in range(1, NCHUNK):
                nc.sync.dma_start(xb[0][:, n], xT_d[:, 0, n])
            nc.sync.dma_start(ident, ident_d[:])
            nc.sync.dma_start(btri, btri_d[:])
            nc.sync.dma_start(sel, sel_d[:])
            nc.sync.dma_start(woutT, woutT_d[:])

            # ---- PE warm-up (HAM: reach K=8/8 before real matmuls) ----
            nc.vector.memset(dummy, 0.0)
            wm = ps_pool.tile([P, 512], f32, tag="ps", name="wm")
            for _ in range(10):
                nc.tensor.matmul(wm, dummy[:, 0:128], dummy[:, 0:512],
                                 start=True, stop=True)

            # ---- per-batch working tiles ----
            qT, kT, vT, vh, attnT, rl2, l2 = [], [], [], [], [], [], []
            for b in range(B):
                qT.append(qkv_pool.tile([P, T], bf16, tag="qT", name=f"qT{b}"))
                kT.append(qkv_pool.tile([P, T], bf16, tag="kT", name=f"kT{b}"))
                vT.append(qkv_pool.tile([P, T], bf16, tag="vT", name=f"vT{b}"))
                vh.append(qkv_pool.tile([P, 16, 2, 65], bf16, tag="vh",
                                        name=f"vh{b}"))
                attnT.append(attnT_pool.tile([P, T], bf16, tag="attnT",
                                             name=f"attnT{b}"))
                rl2.append(rl_pool.tile([65, T], f32, tag="rl2",
                                        name=f"rl2{b}"))
                l2.append(rl_pool.tile([65, T], f32, tag="l2",
                                       name=f"l2{b}"))
                # rows 1-63 are never written; keep them finite for recip
                nc.vector.memset(l2[b], 1.0)
                # ones column (col 64) -> PV row 64 = softmax denominator
                nc.vector.memset(vh[b][:, :, :, 64], 1.0)

            # ---------- emission units ----------
            def emit_qkv(b, n, fi, copy_fn):
                """One projection (fi: 0=q,1=k,2=v) for token chunk n."""
                dest = (qT, kT, vT)[fi][b]
                nsl = slice(n * CH, (n + 1) * CH)
                pss = ps_pool.tile([P, 512], f32, tag="ps", name="pss")
                for c in range(8):
                    nc.tensor.matmul(
                        pss, wqkvT[:, c, fi * 128:(fi + 1) * 128],
                        xb[b][:, n, c, :], start=(c == 0), stop=(c == 7))
                copy_fn(dest[:, nsl], pss)

            def emit_transposes(b, n):
                """PE-transpose V token chunk n (4 t-blocks) into vh."""
                tp = ps_pool.tile([P, 4, P], bf16, tag="ps", name="tp")
                for j in range(4):
                    tb = 4 * n + j
                    nc.tensor.transpose(
                        tp[:, j, :], vT[b][:, tb * 128:(tb + 1) * 128], ident)
                for h in range(2):
                    nc.vector.tensor_copy(
                        vh[b][:, 4 * n:4 * n + 4, h, 0:64],
                        tp[:, :, h * 64:(h + 1) * 64])

            def emit_oproj(b, tb):
                ps_a = ps_pool.tile([P, 512], f32, tag="ps", name="opa")
                ps_b = ps_pool.tile([P, 512], f32, tag="ps", name="opb")
                tsl = slice(tb * 128, (tb + 1) * 128)
                nc.tensor.matmul(ps_a, attnT[b][:, tsl], woutT[:, 0:512],
                                 start=True, stop=True)
                nc.tensor.matmul(ps_b, attnT[b][:, tsl], woutT[:, 512:1024],
                                 start=True, stop=True)
                osb = osb_pool.tile([P, C], f32, tag="osb")
                # keep batch-1 attention's scalar queue pure-exp: route both
                # halves to vector there; the tail (tb>=12) has scalar free
                if b == 0 or tb >= 12:
                    nc.scalar.copy(osb[:, 0:512], ps_a)
                else:
                    nc.vector.tensor_copy(osb[:, 0:512], ps_a)
                nc.vector.tensor_copy(osb[:, 512:1024], ps_b)
                nc.gpsimd.dma_start(
                    out_d[(b * T + tb * 128):(b * T + (tb + 1) * 128), :], osb)

            def emit_rbnorm(b, qc):
                qsl = slice(qc * 512, (qc + 1) * 512)
                rb = ps_pool.tile([P, 512], f32, tag="ps", name="rb")
                nc.tensor.matmul(rb, sel[:, :], rl2[b][:, qsl],
                                 start=True, stop=True)
                nc.vector.tensor_mul(attnT[b][:, qsl], attnT[b][:, qsl], rb)

            # filler queue: popped one unit per site inside the attention loop
            F = deque()

            def pop_F():
                if F:
                    F.popleft()()

            # filler order: V-transposes and remaining QKV chunks first (they
            # have deadlines inside batch-0 attention), then batch 1's QKV,
            # then (appended later, at qc ends) rbnorm+oproj units.
            F.append(lambda: emit_transposes(0, 0))
            for n in range(1, NCHUNK):
                F.append(lambda n=n: nc.sync.dma_start(xb[1][:, n - 1],
                                                       xT_d[:, 1, n - 1]))
                for fi in (1, 0, 2):
                    F.append(lambda b=0, n=n, fi=fi:
                             emit_qkv(b, n, fi, nc.vector.tensor_copy))
                F.append(lambda n=n: emit_transposes(0, n))
            F.append(lambda: nc.sync.dma_start(xb[1][:, NCHUNK - 1],
                                               xT_d[:, 1, NCHUNK - 1]))
            for n in range(NCHUNK):
                for fi in (1, 0, 2):
                    F.append(lambda b=1, n=n, fi=fi:
                             emit_qkv(b, n, fi, nc.vector.tensor_copy))
                F.append(lambda n=n: emit_transposes(1, n))

            # ---------- lead-in: batch 0 token chunk 0 ----------
            for fi in (1, 0, 2):
                emit_qkv(0, 0, fi, nc.scalar.copy)

            # ---------- attention ----------
            pending_drain = [None]

            def make_drain(b, qc):
                qsl = slice(qc * 512, (qc + 1) * 512)

                def drain(pv):
                    # reciprocal_approx_* requires partition base 0: bounce
                    # the two denominator rows into l2 first.
                    for h in range(2):
                        nc.vector.tensor_copy(l2[b][64 * h:64 * h + 1, qsl],
                                              pv[h][64:65, :])
                    nc.vector.reciprocal_approx_fast(rl2[b][:, qsl],
                                                     l2[b][:, qsl])
                    if b == 0:
                        nc.scalar.copy(attnT[b][0:64, qsl], pv[0][0:64, :])
                    else:
                        nc.vector.tensor_copy(attnT[b][0:64, qsl],
                                              pv[0][0:64, :])
                    nc.vector.tensor_copy(attnT[b][64:128, qsl],
                                          pv[1][0:64, :])
                    for tb in range(4 * qc + 3, 4 * qc - 1, -1):
                        F.appendleft(lambda b=b, tb=tb: emit_oproj(b, tb))
                    F.appendleft(lambda: emit_rbnorm(b, qc))
                return drain

            def attention(b):
                for qc in range(4):
                    nk = 4 * qc + 4
                    qsl = slice(qc * 512, (qc + 1) * 512)
                    pv = [pv_pool.tile([P, 512], f32, tag=f"pv{h}",
                                       name=f"pv{h}")
                          for h in range(2)]
                    prev = None
                    for g0 in range(0, nk, 2):
                        kbs = [g0, g0 + 1]
                        st = [st_pool.tile([P, 2, 512], f32, tag="st",
                                           name=f"st{h}")
                              for h in range(2)]
                        pt = [pt_pool.tile([P, 2, 512], bf16, tag="pt",
                                           name=f"pt{h}")
                              for h in range(2)]
                        # scores, heads interleaved for PE row-packing;
                        # diagonal blocks get the causal mask accumulated in
                        # as a -1e4 upper-triangle bias matmul (exp -> 0)
                        for j, kb in enumerate(kbs):
                            diag = kb >= 4 * qc
                            off = (kb - 4 * qc) * 128
                            for h in range(2):
                                hs = h * 64
                                nc.tensor.matmul(
                                    st[h][:, j, :],
                                    kT[b][hs:hs + 64,
                                          kb * 128:(kb + 1) * 128],
                                    qT[b][hs:hs + 64, qsl],
                                    start=True, stop=not diag)
                            if diag:
                                for h in range(2):
                                    nc.tensor.matmul(
                                        st[h][:, j, off:off + 128],
                                        ident, btri,
                                        start=False, stop=True,
                                        skip_group_check=True)
                        for h in range(2):
                            nc.scalar.activation(pt[h], st[h], Exp,
                                                 scale=SCALE)
                        if g0 == 0 and pending_drain[0] is not None:
                            pending_drain[0]()
                            pending_drain[0] = None
                        pop_F()
                        if prev is not None:
                            emit_pv(b, qc, nk, pv, *prev)
                        pop_F()
                        prev = (pt, kbs)
                    emit_pv(b, qc, nk, pv, *prev)
                    dr = make_drain(b, qc)
                    pending_drain[0] = lambda dr=dr, pv=pv: dr(pv)
                    pop_F()
                    pop_F()

            def emit_pv(b, qc, nk, pv, pt, kbs):
                for h in range(2):
                    for j, kb in enumerate(kbs):
                        off = max(0, (kb - 4 * qc) * 128)
                        nc.tensor.matmul(
                            pv[h][:65, off:512],
                            vh[b][:, kb, h, :],
                            pt[h][:, j, off:512],
                            start=(kb == 0), stop=(kb == nk - 1),
                            skip_group_check=True)

            attention(0)
            attention(1)
            if pending_drain[0] is not None:
                pending_drain[0]()
                pending_drain[0] = None
            while F:
                pop_F()

    nc.compile()
    return nc


def _get_compiled():
    global _COMPILED
    if _COMPILED is None:
        _COMPILED = _build()
    return _COMPILED


def make_core_inputs(x, w_qkv, w_out):
    """Host-side shard prep: returns list of per-core input dicts."""
    xf = np.asarray(x, dtype=np.float32).reshape(BT, C)
    # xT[ci, b, n, co, t] = x[b, n*CH + t, co*128 + ci]
    xT = np.ascontiguousarray(
        xf.T.reshape(8, P, B, NCHUNK, CH).transpose(1, 2, 3, 0, 4)
    ).astype(_bf16)

    btri = np.zeros((P, P), dtype=np.float32)
    kk, qq = np.meshgrid(np.arange(P), np.arange(P), indexing="ij")
    btri[kk > qq] = -1e4
    btri = btri.astype(_bf16)

    sel = np.zeros((65, P), dtype=np.float32)
    sel[0, 0:64] = 1.0
    sel[64, 64:128] = 1.0

    ident = np.eye(P, dtype=_bf16)

    w_qkv = np.asarray(w_qkv, dtype=np.float32)
    w_out = np.asarray(w_out, dtype=np.float32)

    ins = []
    for core in range(NCORES):
        r0 = 2 * core * D
        wsel = np.concatenate([
            w_qkv[r0:r0 + 128],
            w_qkv[C + r0:C + r0 + 128],
            w_qkv[2 * C + r0:2 * C + r0 + 128],
        ], axis=0)  # [384, 1024]
        wqkvT = np.ascontiguousarray(
            wsel.T.reshape(8, P, 384).transpose(1, 0, 2)).astype(_bf16)
        woutT = np.ascontiguousarray(
            w_out[:, core * P:(core + 1) * P].T).astype(_bf16)
        ins.append({
            "xT": xT,
            "wqkvT": wqkvT,
            "woutT": woutT,
            "btri": btri,
            "sel": sel,
            "ident": ident,
        })
    return ins


def kernel(x, w_qkv, w_out):
    global LAST_RESULTS
    from concourse.bass_utils import run_bass_kernel_spmd

    nc = _get_compiled()
    ins = make_core_inputs(x, w_qkv, w_out)
    trace = bool(os.environ.get("KERNEL_TRACE"))
    res = run_bass_kernel_spmd(nc, ins, core_ids=list(range(NCORES)),
                               trace=trace)
    LAST_RESULTS = res
    out = np.zeros((BT, C), dtype=np.float32)
    for r in res.results:
        out += r["out"]
    return out.reshape(B, T, C)


# revision 13
# speedup vs baseline: 1.2756x; 1.0147x over previous
"""Causal self-attention (B=2, T=2048, C=1024, 16 heads x 64) on 8 TRN2 cores.

Sharding: tensor-parallel over heads (2 heads/core). Each core computes its
heads' QKV projection, causal attention, and a partial output projection
(contraction over its 128 attn columns); the host sums the 8 partials.

v3 design notes (HAM-warmth + engine-balance rewrite of v2):
  - PE warm-up: 6 dummy matmuls at t~0 so the HAM clock gate reaches 8/8
    (2.4 GHz) right as the first real matmuls issue.
  - x DMA'd in 4 token-chunks per batch; QKV runs token-chunk-pipelined
    (c-inner accumulation, one PSUM bank per chunk) so attention for batch 0
    starts ~6us in. Remaining chunks + all of batch 1's QKV/V-transposes are
    emitted as PE filler units INSIDE the attention loop (tensor-queue FIFO =
    schedule), keeping the PE dense so HAM never re-throttles.
  - Attention software pipeline: scores(g) -> exp(g) [scalar] -> mask(g)
    [gpsimd] -> PV(g), with PV emitted one group behind scores so the PE
    never head-blocks on the exp stream (the scalar engine's ~91us of exp is
    the attention-phase clock; it must never starve).
  - PV keeps the ones-column trick (ones column last -> PSUM row 64; DVE/ACT
    partition bases must be quadrant-aligned, and reciprocal_approx_* needs
    base 0, hence the l2 bounce).
  - Engine balance: exp on scalar; masks on gpsimd; PSUM evacuations split
    scalar/vector so neither exceeds the tensor engine's ~106us of matmul.
  - Drains of qc N are emitted after qc N+1's first exps so the scalar queue
    never head-blocks waiting for PV.
"""

import os
from collections import deque

import numpy as np
import ml_dtypes

B = 2
T = 2048
C = 1024
N_HEADS = 16
D = 64
NCORES = 8
P = 128
BT = B * T
SCALE = D ** -0.5
NCHUNK = 4          # token chunks per batch for QKV pipeline
CH = T // NCHUNK    # 512

_bf16 = ml_dtypes.bfloat16

_COMPILED = None
LAST_RESULTS = None  # stashed BassKernelResults for test harness introspection


def _build():
    import concourse.bass as bass
    import concourse.mybir as mybir
    import concourse.tile as tile
    from concourse import bacc

    f32 = mybir.dt.float32
    bf16 = mybir.dt.bfloat16

    nc = bacc.Bacc("TRN2", target_bir_lowering=False, debug=False,
                   num_devices=NCORES)

    xT_d = nc.dram_tensor("xT", [P, B, NCHUNK, 8, CH], bf16,
                          kind="ExternalInput")
    wqkvT_d = nc.dram_tensor("wqkvT", [P, 8, 384], bf16, kind="ExternalInput")
    woutT_d = nc.dram_tensor("woutT", [P, C], bf16, kind="ExternalInput")
    btri_d = nc.dram_tensor("btri", [P, P], bf16, kind="ExternalInput")
    sel_d = nc.dram_tensor("sel", [65, P], bf16, kind="ExternalInput")
    ident_d = nc.dram_tensor("ident", [P, P], bf16, kind="ExternalInput")
    out_d = nc.dram_tensor("out", [BT, C], bf16, kind="ExternalOutput")

    Exp = mybir.ActivationFunctionType.Exp

    with tile.TileContext(nc) as tc:
        with (
            tc.tile_pool(name="const", bufs=1) as const_pool,
            tc.tile_pool(name="xb", bufs=2) as xb_pool,
            tc.tile_pool(name="qkv", bufs=2) as qkv_pool,
            tc.tile_pool(name="pt", bufs=4) as pt_pool,
            tc.tile_pool(name="attnT", bufs=2) as attnT_pool,
            tc.tile_pool(name="rl", bufs=2) as rl_pool,
            tc.tile_pool(name="osb", bufs=3) as osb_pool,
            tc.tile_pool(name="st", bufs=2, space="PSUM") as st_pool,
            tc.tile_pool(name="pv", bufs=1, space="PSUM") as pv_pool,
            tc.tile_pool(name="ps", bufs=2, space="PSUM") as ps_pool,
        ):
            # ---- constants ----
            wqkvT = const_pool.tile([P, 8, 384], bf16, tag="wqkvT")
            woutT = const_pool.tile([P, C], bf16, tag="woutT")
            btri = const_pool.tile([P, P], bf16, tag="btri")
            sel = const_pool.tile([65, P], bf16, tag="sel")
            ident = const_pool.tile([P, P], bf16, tag="ident")
            dummy = const_pool.tile([P, 512], bf16, tag="dummy")
            # K weights + first x chunk first: they gate the first real matmul
            nc.sync.dma_start(wqkvT[:, :, 128:256], wqkvT_d[:, :, 128:256])

            # ---- x, token-chunked (batch 1 chunks deferred into the
            # attention filler stream to keep early HBM read BW for batch 0)
            xb = []
            for b in range(B):
                xt = xb_pool.tile([P, NCHUNK, 8, CH], bf16, tag="xb",
                                  name=f"xb{b}")
                xb.append(xt)
            nc.sync.dma_start(xb[0][:, 0], xT_d[:, 0, 0])
            nc.sync.dma_start(wqkvT[:, :, 0:128], wqkvT_d[:, :, 0:128])
            nc.sync.dma_start(wqkvT[:, :, 256:384], wqkvT_d[:, :, 256:384])
            for n in range(1, NCHUNK):
                nc.sync.dma_start(xb[0][:, n], xT_d[:, 0, n])
            nc.sync.dma_start(ident, ident_d[:])
            nc.sync.dma_start(btri, btri_d[:])
            nc.sync.dma_start(sel, sel_d[:])
            nc.sync.dma_start(woutT, woutT_d[:])

            # ---- PE warm-up (HAM: reach K=8/8 before real matmuls) ----
            nc.vector.memset(dummy, 0.0)
            wm = ps_pool.tile([P, 512], f32, tag="ps", name="wm")
            for _ in range(10):
                nc.tensor.matmul(wm, dummy[:, 0:128], dummy[:, 0:512],
                                 start=True, stop=True)

            # ---- per-batch working tiles ----
            qT, kT, vT, vh, attnT, rl2, l2, rl2b = ([], [], [], [], [], [],
                                                    [], [])
            for b in range(B):
                qT.append(qkv_pool.tile([P, T], bf16, tag="qT", name=f"qT{b}"))
                kT.append(qkv_pool.tile([P, T], bf16, tag="kT", name=f"kT{b}"))
                vT.append(qkv_pool.tile([P, T], bf16, tag="vT", name=f"vT{b}"))
                vh.append(qkv_pool.tile([P, 16, 2, 65], bf16, tag="vh",
                                        name=f"vh{b}"))
                attnT.append(attnT_pool.tile([P, T], bf16, tag="attnT",
                                             name=f"attnT{b}"))
                rl2.append(rl_pool.tile([65, T], f32, tag="rl2",
                                        name=f"rl2{b}"))
                l2.append(rl_pool.tile([65, T], f32, tag="l2",
                                       name=f"l2{b}"))
                rl2b.append(rl_pool.tile([65, T], bf16, tag="rl2b",
                                         name=f"rl2b{b}"))
                # rows 1-63 are never written; keep them finite for recip
                nc.vector.memset(l2[b], 1.0)
                # ones column (col 64) -> PV row 64 = softmax denominator
                nc.vector.memset(vh[b][:, :, :, 64], 1.0)

            # ---------- emission units ----------
            def emit_qkv(b, n, fi, copy_fn):
                """One projection (fi: 0=q,1=k,2=v) for token chunk n."""
                dest = (qT, kT, vT)[fi][b]
                nsl = slice(n * CH, (n + 1) * CH)
                pss = ps_pool.tile([P, 512], f32, tag="ps", name="pss")
                for c in range(8):
                    nc.tensor.matmul(
                        pss, wqkvT[:, c, fi * 128:(fi + 1) * 128],
                        xb[b][:, n, c, :], start=(c == 0), stop=(c == 7))
                copy_fn(dest[:, nsl], pss)

            def emit_transposes(b, n):
                """PE-transpose V token chunk n (4 t-blocks) into vh."""
                tp = ps_pool.tile([P, 4, P], bf16, tag="ps", name="tp")
                for j in range(4):
                    tb = 4 * n + j
                    nc.tensor.transpose(
                        tp[:, j, :], vT[b][:, tb * 128:(tb + 1) * 128], ident)
                for h in range(2):
                    nc.vector.tensor_copy(
                        vh[b][:, 4 * n:4 * n + 4, h, 0:64],
                        tp[:, :, h * 64:(h + 1) * 64])

            def emit_oproj(b, tb):
                ps_a = ps_pool.tile([P, 512], f32, tag="ps", name="opa")
                ps_b = ps_pool.tile([P, 512], f32, tag="ps", name="opb")
                tsl = slice(tb * 128, (tb + 1) * 128)
                nc.tensor.matmul(ps_a, attnT[b][:, tsl], woutT[:, 0:512],
                                 start=True, stop=True)
                nc.tensor.matmul(ps_b, attnT[b][:, tsl], woutT[:, 512:1024],
                                 start=True, stop=True)
                osb = osb_pool.tile([P, C], bf16, tag="osb")
                # keep batch-1 attention's scalar queue pure-exp: route both
                # halves to vector there; the tail (tb>=12) has scalar free
                if b == 0 or tb >= 12:
                    nc.scalar.copy(osb[:, 0:512], ps_a)
                else:
                    nc.vector.tensor_copy(osb[:, 0:512], ps_a)
                nc.vector.tensor_copy(osb[:, 512:1024], ps_b)
                nc.gpsimd.dma_start(
                    out_d[(b * T + tb * 128):(b * T + (tb + 1) * 128), :], osb)

            def emit_rbnorm(b, qc):
                qsl = slice(qc * 512, (qc + 1) * 512)
                rb = ps_pool.tile([P, 512], f32, tag="ps", name="rb")
                nc.tensor.matmul(rb, sel[:, :], rl2b[b][:, qsl],
                                 start=True, stop=True)
                nc.vector.tensor_mul(attnT[b][:, qsl], attnT[b][:, qsl], rb)

            # filler queue: popped one unit per site inside the attention loop
            F = deque()

            def pop_F():
                if F:
                    F.popleft()()

            # filler order: V-transposes and remaining QKV chunks first (they
            # have deadlines inside batch-0 attention), then batch 1's QKV,
            # then (appended later, at qc ends) rbnorm+oproj units.
            F.append(lambda: emit_transposes(0, 0))
            for n in range(1, NCHUNK):
                F.append(lambda n=n: nc.sync.dma_start(xb[1][:, n - 1],
                                                       xT_d[:, 1, n - 1]))
                for fi in (1, 0, 2):
                    F.append(lambda b=0, n=n, fi=fi:
                             emit_qkv(b, n, fi, nc.vector.tensor_copy))
                F.append(lambda n=n: emit_transposes(0, n))
            F.append(lambda: nc.sync.dma_start(xb[1][:, NCHUNK - 1],
                                               xT_d[:, 1, NCHUNK - 1]))
            for n in range(NCHUNK):
                for fi in (1, 0, 2):
                    F.append(lambda b=1, n=n, fi=fi:
                             emit_qkv(b, n, fi, nc.vector.tensor_copy))
                F.append(lambda n=n: emit_transposes(1, n))

            # ---------- lead-in: batch 0 token chunk 0 ----------
            for fi in (1, 0, 2):
                emit_qkv(0, 0, fi, nc.scalar.copy)

            # ---------- attention ----------
            pending_drain = [None]

            def make_drain(b, qc):
                qsl = slice(qc * 512, (qc + 1) * 512)

                def drain(pv):
                    # reciprocal_approx_* requires partition base 0: bounce
                    # the two denominator rows into l2 first.
                    for h in range(2):
                        nc.vector.tensor_copy(l2[b][64 * h:64 * h + 1, qsl],
                                              pv[h][64:65, :])
                    nc.vector.reciprocal_approx_fast(rl2[b][:, qsl],
                                                     l2[b][:, qsl])
                    nc.vector.tensor_copy(rl2b[b][:, qsl], rl2[b][:, qsl])
                    if b == 0:
                        nc.scalar.copy(attnT[b][0:64, qsl], pv[0][0:64, :])
                    else:
                        nc.vector.tensor_copy(attnT[b][0:64, qsl],
                                              pv[0][0:64, :])
                    nc.vector.tensor_copy(attnT[b][64:128, qsl],
                                          pv[1][0:64, :])
                    for tb in range(4 * qc + 3, 4 * qc - 1, -1):
                        F.appendleft(lambda b=b, tb=tb: emit_oproj(b, tb))
                    F.appendleft(lambda: emit_rbnorm(b, qc))
                return drain

            def attention(b):
                for qc in range(4):
                    nk = 4 * qc + 4
                    qsl = slice(qc * 512, (qc + 1) * 512)
                    pv = [pv_pool.tile([P, 512], f32, tag=f"pv{h}",
                                       name=f"pv{h}")
                          for h in range(2)]
                    prev = None
                    for g0 in range(0, nk, 2):
                        kbs = [g0, g0 + 1]
                        st = [st_pool.tile([P, 2, 512], f32, tag="st",
                                           name=f"st{h}")
                              for h in range(2)]
                        pt = [pt_pool.tile([P, 2, 512], bf16, tag="pt",
                                           name=f"pt{h}")
                              for h in range(2)]
                        # scores, heads interleaved for PE row-packing;
                        # diagonal blocks get the causal mask accumulated in
                        # as a -1e4 upper-triangle bias matmul (exp -> 0)
                        for j, kb in enumerate(kbs):
                            diag = kb >= 4 * qc
                            off = (kb - 4 * qc) * 128
                            nc.tensor.ldweights(
                                kT[b][64:128, kb * 128:(kb + 1) * 128],
                                tile_position=(64, 0))
                            for h in range(2):
                                hs = h * 64
                                nc.tensor.matmul(
                                    st[h][:, j, :],
                                    kT[b][hs:hs + 64,
                                          kb * 128:(kb + 1) * 128],
                                    qT[b][hs:hs + 64, qsl],
                                    start=True, stop=not diag)
                            if diag:
                                for h in range(2):
                                    nc.tensor.matmul(
                                        st[h][:, j, off:off + 128],
                                        ident, btri,
                                        start=False, stop=True,
                                        skip_group_check=True)
                        for h in range(2):
                            nc.scalar.activation(pt[h], st[h], Exp,
                                                 scale=SCALE)
                        if g0 == 0 and pending_drain[0] is not None:
                            pending_drain[0]()
                            pending_drain[0] = None
                        pop_F()
                        if prev is not None:
                            emit_pv(b, qc, nk, pv, *prev)
                        pop_F()
                        prev = (pt, kbs)
                    emit_pv(b, qc, nk, pv, *prev)
                    dr = make_drain(b, qc)
                    pending_drain[0] = lambda dr=dr, pv=pv: dr(pv)
                    pop_F()
                    pop_F()

            def emit_pv(b, qc, nk, pv, pt, kbs):
                for h in range(2):
                    for j, kb in enumerate(kbs):
                        off = max(0, (kb - 4 * qc) * 128)
                        nc.tensor.matmul(
                            pv[h][:65, off:512],
                            vh[b][:, kb, h, :],
                            pt[h][:, j, off:512],
                            start=(kb == 0), stop=(kb == nk - 1),
                            skip_group_check=True)

            attention(0)
            attention(1)
            if pending_drain[0] is not None:
                pending_drain[0]()
                pending_drain[0] = None
            while F:
                pop_F()

    nc.compile()
    return nc


def _get_compiled():
    global _COMPILED
    if _COMPILED is None:
        _COMPILED = _build()
    return _COMPILED


def make_core_inputs(x, w_qkv, w_out):
    """Host-side shard prep: returns list of per-core input dicts."""
    xf = np.asarray(x, dtype=np.float32).reshape(BT, C)
    # xT[ci, b, n, co, t] = x[b, n*CH + t, co*128 + ci]
    xT = np.ascontiguousarray(
        xf.T.reshape(8, P, B, NCHUNK, CH).transpose(1, 2, 3, 0, 4)
    ).astype(_bf16)

    btri = np.zeros((P, P), dtype=np.float32)
    kk, qq = np.meshgrid(np.arange(P), np.arange(P), indexing="ij")
    btri[kk > qq] = -1e4
    btri = btri.astype(_bf16)

    sel = np.zeros((65, P), dtype=_bf16)
    sel[0, 0:64] = 1.0
    sel[64, 64:128] = 1.0

    ident = np.eye(P, dtype=_bf16)

    w_qkv = np.asarray(w_qkv, dtype=np.float32)
    w_out = np.asarray(w_out, dtype=np.float32)

    ins = []
    for core in range(NCORES):
        r0 = 2 * core * D
        wsel = np.concatenate([
            w_qkv[r0:r0 + 128],
            w_qkv[C + r0:C + r0 + 128],
            w_qkv[2 * C + r0:2 * C + r0 + 128],
        ], axis=0)  # [384, 1024]
        wqkvT = np.ascontiguousarray(
            wsel.T.reshape(8, P, 384).transpose(1, 0, 2)).astype(_bf16)
        woutT = np.ascontiguousarray(
            w_out[:, core * P:(core + 1) * P].T).astype(_bf16)
        ins.append({
            "xT": xT,
            "wqkvT": wqkvT,
            "woutT": woutT,
            "btri": btri,
            "sel": sel,
            "ident": ident,
        })
    return ins


def kernel(x, w_qkv, w_out):
    global LAST_RESULTS
    from concourse.bass_utils import run_bass_kernel_spmd

    nc = _get_compiled()
    ins = make_core_inputs(x, w_qkv, w_out)
    trace = bool(os.environ.get("KERNEL_TRACE"))
    res = run_bass_kernel_spmd(nc, ins, core_ids=list(range(NCORES)),
                               trace=trace)
    LAST_RESULTS = res
    out = np.zeros((BT, C), dtype=np.float32)
    for r in res.results:
        out += r["out"]
    return out.reshape(B, T, C)


# revision 14
# speedup vs baseline: 1.3096x; 1.0266x over previous
"""Causal self-attention (B=2, T=2048, C=1024, 16 heads x 64) on 8 TRN2 cores.

Sharding: tensor-parallel over heads (2 heads/core). Each core computes its
heads' QKV projection, causal attention, and a partial output projection
(contraction over its 128 attn columns); the host sums the 8 partials.

v3 design notes (HAM-warmth + engine-balance rewrite of v2):
  - PE warm-up: 6 dummy matmuls at t~0 so the HAM clock gate reaches 8/8
    (2.4 GHz) right as the first real matmuls issue.
  - x DMA'd in 4 token-chunks per batch; QKV runs token-chunk-pipelined
    (c-inner accumulation, one PSUM bank per chunk) so attention for batch 0
    starts ~6us in. Remaining chunks + all of batch 1's QKV/V-transposes are
    emitted as PE filler units INSIDE the attention loop (tensor-queue FIFO =
    schedule), keeping the PE dense so HAM never re-throttles.
  - Attention software pipeline: scores(g) -> exp(g) [scalar] -> mask(g)
    [gpsimd] -> PV(g), with PV emitted one group behind scores so the PE
    never head-blocks on the exp stream (the scalar engine's ~91us of exp is
    the attention-phase clock; it must never starve).
  - PV keeps the ones-column trick (ones column last -> PSUM row 64; DVE/ACT
    partition bases must be quadrant-aligned, and reciprocal_approx_* needs
    base 0, hence the l2 bounce).
  - Engine balance: exp on scalar; masks on gpsimd; PSUM evacuations split
    scalar/vector so neither exceeds the tensor engine's ~106us of matmul.
  - Drains of qc N are emitted after qc N+1's first exps so the scalar queue
    never head-blocks waiting for PV.
"""

import os
from collections import deque

import numpy as np
import ml_dtypes

B = 2
T = 2048
C = 1024
N_HEADS = 16
D = 64
NCORES = 8
P = 128
BT = B * T
SCALE = D ** -0.5
NCHUNK = 4          # token chunks per batch for QKV pipeline
CH = T // NCHUNK    # 512

_bf16 = ml_dtypes.bfloat16

_COMPILED = None
LAST_RESULTS = None  # stashed BassKernelResults for test harness introspection


def _build():
    import concourse.bass as bass
    import concourse.mybir as mybir
    import concourse.tile as tile
    from concourse import bacc

    f32 = mybir.dt.float32
    bf16 = mybir.dt.bfloat16

    nc = bacc.Bacc("TRN2", target_bir_lowering=False, debug=False,
                   num_devices=NCORES)

    xT_d = nc.dram_tensor("xT", [P, B, NCHUNK, 8, CH], bf16,
                          kind="ExternalInput")
    wqkvT_d = nc.dram_tensor("wqkvT", [P, 8, 384], bf16, kind="ExternalInput")
    woutT_d = nc.dram_tensor("woutT", [P, C], bf16, kind="ExternalInput")
    btri_d = nc.dram_tensor("btri", [P, P], bf16, kind="ExternalInput")
    sel_d = nc.dram_tensor("sel", [65, P], bf16, kind="ExternalInput")
    ident_d = nc.dram_tensor("ident", [P, P], bf16, kind="ExternalInput")
    out_d = nc.dram_tensor("out", [BT, C], bf16, kind="ExternalOutput")

    Exp = mybir.ActivationFunctionType.Exp

    with tile.TileContext(nc) as tc:
        with (
            tc.tile_pool(name="const", bufs=1) as const_pool,
            tc.tile_pool(name="xb", bufs=2) as xb_pool,
            tc.tile_pool(name="qkv", bufs=2) as qkv_pool,
            tc.tile_pool(name="pt", bufs=4) as pt_pool,
            tc.tile_pool(name="attnT", bufs=2) as attnT_pool,
            tc.tile_pool(name="rl", bufs=2) as rl_pool,
            tc.tile_pool(name="osb", bufs=3) as osb_pool,
            tc.tile_pool(name="st", bufs=2, space="PSUM") as st_pool,
            tc.tile_pool(name="pv", bufs=1, space="PSUM") as pv_pool,
            tc.tile_pool(name="ps", bufs=2, space="PSUM") as ps_pool,
        ):
            # ---- constants ----
            wqkvT = const_pool.tile([P, 8, 384], bf16, tag="wqkvT")
            woutT = const_pool.tile([P, C], bf16, tag="woutT")
            btri = const_pool.tile([P, P], bf16, tag="btri")
            sel = const_pool.tile([65, P], bf16, tag="sel")
            ident = const_pool.tile([P, P], bf16, tag="ident")
            dummy = const_pool.tile([P, 512], bf16, tag="dummy")
            # K weights + first x chunk first: they gate the first real matmul
            nc.sync.dma_start(wqkvT[:, :, 128:256], wqkvT_d[:, :, 128:256])

            # ---- x, token-chunked (batch 1 chunks deferred into the
            # attention filler stream to keep early HBM read BW for batch 0)
            xb = []
            for b in range(B):
                xt = xb_pool.tile([P, NCHUNK, 8, CH], bf16, tag="xb",
                                  name=f"xb{b}")
                xb.append(xt)
            nc.sync.dma_start(xb[0][:, 0], xT_d[:, 0, 0])
            nc.sync.dma_start(wqkvT[:, :, 0:128], wqkvT_d[:, :, 0:128])
            nc.sync.dma_start(wqkvT[:, :, 256:384], wqkvT_d[:, :, 256:384])
            for n in range(1, NCHUNK):
                nc.sync.dma_start(xb[0][:, n], xT_d[:, 0, n])
            nc.sync.dma_start(ident, ident_d[:])
            nc.sync.dma_start(btri, btri_d[:])
            nc.sync.dma_start(sel, sel_d[:])
            nc.sync.dma_start(woutT, woutT_d[:])

            # ---- PE warm-up (HAM: reach K=8/8 before real matmuls) ----
            nc.vector.memset(dummy, 0.0)
            wm = ps_pool.tile([P, 512], f32, tag="ps", name="wm")
            for _ in range(10):
                nc.tensor.matmul(wm, dummy[:, 0:128], dummy[:, 0:512],
                                 start=True, stop=True)

            # ---- per-batch working tiles ----
            qT, kT, vT, vh, attnT, rl2, l2, rl2b = ([], [], [], [], [], [],
                                                    [], [])
            for b in range(B):
                qT.append(qkv_pool.tile([P, T], bf16, tag="qT", name=f"qT{b}"))
                kT.append(qkv_pool.tile([P, T], bf16, tag="kT", name=f"kT{b}"))
                vT.append(qkv_pool.tile([P, T], bf16, tag="vT", name=f"vT{b}"))
                vh.append(qkv_pool.tile([P, 16, 2, 65], bf16, tag="vh",
                                        name=f"vh{b}"))
                attnT.append(attnT_pool.tile([P, T], bf16, tag="attnT",
                                             name=f"attnT{b}"))
                rl2.append(rl_pool.tile([65, T], f32, tag="rl2",
                                        name=f"rl2{b}"))
                l2.append(rl_pool.tile([65, T], f32, tag="l2",
                                       name=f"l2{b}"))
                rl2b.append(rl_pool.tile([65, T], bf16, tag="rl2b",
                                         name=f"rl2b{b}"))
                # rows 1-63 are never written; keep them finite for recip
                nc.vector.memset(l2[b], 1.0)
                # ones column (col 64) -> PV row 64 = softmax denominator
                nc.vector.memset(vh[b][:, :, :, 64], 1.0)

            # ---------- emission units ----------
            def emit_qkv(b, n, fi, copy_fn):
                """One projection (fi: 0=q,1=k,2=v) for token chunk n."""
                dest = (qT, kT, vT)[fi][b]
                nsl = slice(n * CH, (n + 1) * CH)
                pss = ps_pool.tile([P, 512], f32, tag="ps", name="pss")
                for c in range(8):
                    nc.tensor.matmul(
                        pss, wqkvT[:, c, fi * 128:(fi + 1) * 128],
                        xb[b][:, n, c, :], start=(c == 0), stop=(c == 7))
                copy_fn(dest[:, nsl], pss)

            def emit_transposes(b, n):
                """PE-transpose V token chunk n (4 t-blocks) into vh."""
                tp = ps_pool.tile([P, 4, P], bf16, tag="ps", name="tp")
                for j in range(4):
                    tb = 4 * n + j
                    nc.tensor.transpose(
                        tp[:, j, :], vT[b][:, tb * 128:(tb + 1) * 128], ident)
                for h in range(2):
                    nc.vector.tensor_copy(
                        vh[b][:, 4 * n:4 * n + 4, h, 0:64],
                        tp[:, :, h * 64:(h + 1) * 64])

            def emit_oproj(b, tb):
                ps_a = ps_pool.tile([P, 512], f32, tag="ps", name="opa")
                ps_b = ps_pool.tile([P, 512], f32, tag="ps", name="opb")
                tsl = slice(tb * 128, (tb + 1) * 128)
                nc.tensor.matmul(ps_a, attnT[b][:, tsl], woutT[:, 0:512],
                                 start=True, stop=True)
                nc.tensor.matmul(ps_b, attnT[b][:, tsl], woutT[:, 512:1024],
                                 start=True, stop=True)
                osb = osb_pool.tile([P, C], bf16, tag="osb")
                # keep batch-1 attention's scalar queue pure-exp: route both
                # halves to vector there; the tail (tb>=12) has scalar free
                if b == 0 or tb >= 12:
                    nc.scalar.copy(osb[:, 0:512], ps_a)
                else:
                    nc.vector.tensor_copy(osb[:, 0:512], ps_a)
                nc.vector.tensor_copy(osb[:, 512:1024], ps_b)
                nc.gpsimd.dma_start(
                    out_d[(b * T + tb * 128):(b * T + (tb + 1) * 128), :], osb)

            def emit_rbnorm(b, qc):
                qsl = slice(qc * 512, (qc + 1) * 512)
                rb = ps_pool.tile([P, 512], f32, tag="ps", name="rb")
                nc.tensor.matmul(rb, sel[:, :], rl2b[b][:, qsl],
                                 start=True, stop=True)
                nc.vector.tensor_mul(attnT[b][:, qsl], attnT[b][:, qsl], rb)

            # filler queue: popped one unit per site inside the attention loop
            F = deque()

            def pop_F():
                if F:
                    F.popleft()()

            # filler order: V-transposes and remaining QKV chunks first (they
            # have deadlines inside batch-0 attention), then batch 1's QKV,
            # then (appended later, at qc ends) rbnorm+oproj units.
            F.append(lambda: nc.sync.dma_start(xb[1][:, 0], xT_d[:, 1, 0]))
            F.append(lambda: emit_transposes(0, 0))
            for n in range(1, NCHUNK):
                F.append(lambda n=n: nc.sync.dma_start(xb[1][:, n],
                                                       xT_d[:, 1, n]))
                for fi in (1, 0, 2):
                    F.append(lambda b=0, n=n, fi=fi:
                             emit_qkv(b, n, fi, nc.vector.tensor_copy))
                F.append(lambda n=n: emit_transposes(0, n))
            for n in range(NCHUNK):
                for fi in (1, 0, 2):
                    F.append(lambda b=1, n=n, fi=fi:
                             emit_qkv(b, n, fi, nc.vector.tensor_copy))
                F.append(lambda n=n: emit_transposes(1, n))

            # ---------- lead-in: batch 0 token chunk 0 ----------
            for fi in (1, 0, 2):
                emit_qkv(0, 0, fi, nc.scalar.copy)

            # ---------- attention ----------
            pending_drain = [None]

            def make_drain(b, qc):
                qsl = slice(qc * 512, (qc + 1) * 512)

                def drain(pv):
                    # reciprocal_approx_* requires partition base 0: bounce
                    # the two denominator rows into l2 first.
                    for h in range(2):
                        nc.vector.tensor_copy(l2[b][64 * h:64 * h + 1, qsl],
                                              pv[h][64:65, :])
                    nc.vector.reciprocal_approx_fast(rl2[b][:, qsl],
                                                     l2[b][:, qsl])
                    nc.vector.tensor_copy(rl2b[b][:, qsl], rl2[b][:, qsl])
                    if b == 0:
                        nc.scalar.copy(attnT[b][0:64, qsl], pv[0][0:64, :])
                    else:
                        nc.vector.tensor_copy(attnT[b][0:64, qsl],
                                              pv[0][0:64, :])
                    nc.vector.tensor_copy(attnT[b][64:128, qsl],
                                          pv[1][0:64, :])
                    for tb in range(4 * qc + 3, 4 * qc - 1, -1):
                        F.appendleft(lambda b=b, tb=tb: emit_oproj(b, tb))
                    F.appendleft(lambda: emit_rbnorm(b, qc))
                return drain

            def attention(b):
                for qc in range(4):
                    nk = 4 * qc + 4
                    qsl = slice(qc * 512, (qc + 1) * 512)
                    pv = [pv_pool.tile([P, 512], f32, tag=f"pv{h}",
                                       name=f"pv{h}")
                          for h in range(2)]
                    prev = None
                    for g0 in range(0, nk, 2):
                        kbs = [g0, g0 + 1]
                        st = [st_pool.tile([P, 2, 512], f32, tag="st",
                                           name=f"st{h}")
                              for h in range(2)]
                        pt = [pt_pool.tile([P, 2, 512], bf16, tag="pt",
                                           name=f"pt{h}")
                              for h in range(2)]
                        # scores, heads interleaved for PE row-packing;
                        # diagonal blocks get the causal mask accumulated in
                        # as a -1e4 upper-triangle bias matmul (exp -> 0)
                        for j, kb in enumerate(kbs):
                            diag = kb >= 4 * qc
                            off = (kb - 4 * qc) * 128
                            for h in range(2):
                                hs = h * 64
                                nc.tensor.matmul(
                                    st[h][:, j, :],
                                    kT[b][hs:hs + 64,
                                          kb * 128:(kb + 1) * 128],
                                    qT[b][hs:hs + 64, qsl],
                                    start=True, stop=not diag)
                            if diag:
                                for h in range(2):
                                    nc.tensor.matmul(
                                        st[h][:, j, off:off + 128],
                                        ident, btri,
                                        start=False, stop=True,
                                        skip_group_check=True)
                        for h in range(2):
                            nc.scalar.activation(pt[h], st[h], Exp,
                                                 scale=SCALE)
                        if g0 == 0 and pending_drain[0] is not None:
                            pending_drain[0]()
                            pending_drain[0] = None
                        pop_F()
                        if prev is not None:
                            emit_pv(b, qc, nk, pv, *prev)
                        pop_F()
                        prev = (pt, kbs)
                    emit_pv(b, qc, nk, pv, *prev)
                    dr = make_drain(b, qc)
                    pending_drain[0] = lambda dr=dr, pv=pv: dr(pv)
                    pop_F()
                    pop_F()

            def emit_pv(b, qc, nk, pv, pt, kbs):
                for h in range(2):
                    for j, kb in enumerate(kbs):
                        off = max(0, (kb - 4 * qc) * 128)
                        nc.tensor.matmul(
                            pv[h][:65, off:512],
                            vh[b][:, kb, h, :],
                            pt[h][:, j, off:512],
                            start=(kb == 0), stop=(kb == nk - 1),
                            skip_group_check=True)

            attention(0)
            attention(1)
            if pending_drain[0] is not None:
                pending_drain[0]()
                pending_drain[0] = None
            while F:
                pop_F()

    nc.compile()
    return nc


def _get_compiled():
    global _COMPILED
    if _COMPILED is None:
        _COMPILED = _build()
    return _COMPILED


def make_core_inputs(x, w_qkv, w_out):
    """Host-side shard prep: returns list of per-core input dicts."""
    xf = np.asarray(x, dtype=np.float32).reshape(BT, C)
    # xT[ci, b, n, co, t] = x[b, n*CH + t, co*128 + ci]
    xT = np.ascontiguousarray(
        xf.T.reshape(8, P, B, NCHUNK, CH).transpose(1, 2, 3, 0, 4)
    ).astype(_bf16)

    btri = np.zeros((P, P), dtype=np.float32)
    kk, qq = np.meshgrid(np.arange(P), np.arange(P), indexing="ij")
    btri[kk > qq] = -1e4
    btri = btri.astype(_bf16)

    sel = np.zeros((65, P), dtype=_bf16)
    sel[0, 0:64] = 1.0
    sel[64, 64:128] = 1.0

    ident = np.eye(P, dtype=_bf16)

    w_qkv = np.asarray(w_qkv, dtype=np.float32)
    w_out = np.asarray(w_out, dtype=np.float32)

    ins = []
    for core in range(NCORES):
        r0 = 2 * core * D
        wsel = np.concatenate([
            w_qkv[r0:r0 + 128],
            w_qkv[C + r0:C + r0 + 128],
            w_qkv[2 * C + r0:2 * C + r0 + 128],
        ], axis=0)  # [384, 1024]
        wqkvT = np.ascontiguousarray(
            wsel.T.reshape(8, P, 384).transpose(1, 0, 2)).astype(_bf16)
        woutT = np.ascontiguousarray(
            w_out[:, core * P:(core + 1) * P].T).astype(_bf16)
        ins.append({
            "xT": xT,
            "wqkvT": wqkvT,
            "woutT": woutT,
            "btri": btri,
            "sel": sel,
            "ident": ident,
        })
    return ins


def kernel(x, w_qkv, w_out):
    global LAST_RESULTS
    from concourse.bass_utils import run_bass_kernel_spmd

    nc = _get_compiled()
    ins = make_core_inputs(x, w_qkv, w_out)
    trace = bool(os.environ.get("KERNEL_TRACE"))
    res = run_bass_kernel_spmd(nc, ins, core_ids=list(range(NCORES)),
                               trace=trace)
    LAST_RESULTS = res
    out = np.zeros((BT, C), dtype=np.float32)
    for r in res.results:
        out += r["out"]
    return out.reshape(B, T, C)
